# revision 2
# baseline (speedup 1.0000x reference)
"""Distributed multi-head attention kernel for 8 TRN2 NeuronCores.

Problem: B=2, S=2048, D=1024, H=16 heads (hd=64).
  qkv = x @ w_qkv.T ; attention per head ; out = attn @ w_out.T

Sharding (no hardware collectives needed):
  core c -> batch b = c // 4, head-group g = c % 4 (heads 4g..4g+3).
  Each core computes a *partial* output projection (its 256 attn channels
  against the full w_out columns); the host sums the 4 partials per batch.

Per-core layout trick: the host pre-transposes x and the weight shards so
every matmul operand arrives with its contraction dim on partitions --
zero on-chip transposes.

Compute dtypes: fp32r (TF32-like, full-rate) for projections, bf16 for the
softmax/PV stage (inputs are well-scaled; PSUM accumulates fp32).

Single unified PSUM pool (8 banks: proj/outproj 2 + scores 4 + PV 2) so
projection, attention and output projection pipeline purely by data deps.
"""

import sys

sys.path.insert(0, "/opt/trn_rl_repo")

import numpy as np
import ml_dtypes

import concourse.bass as bass  # noqa: F401
import concourse.mybir as mybir
import concourse.tile as tile
from concourse import bacc
from concourse.bass_utils import run_bass_kernel_spmd

B, S, D, H = 2, 2048, 1024, 16
HL = 4          # heads per core
HD = 64         # head dim
EL = HL * HD    # local attn channels (256)
N_CORES = 8

f32 = mybir.dt.float32
f32r = mybir.dt.float32r
bf16 = mybir.dt.bfloat16
AF = mybir.ActivationFunctionType

SCALE = 1.0 / (HD ** 0.5)

_CACHE = {}


def build_nc(n_reps=1):
    nc = bacc.Bacc("TRN2", target_bir_lowering=False, debug=False,
                   num_devices=N_CORES)
    xt = nc.dram_tensor("xt", [D, S], bf16, kind="ExternalInput")
    wqk = nc.dram_tensor("wqk", [D, 2 * EL], bf16, kind="ExternalInput")
    wv = nc.dram_tensor("wv", [D, EL], bf16, kind="ExternalInput")
    wo = nc.dram_tensor("wo", [EL, D], f32r, kind="ExternalInput")
    out = nc.dram_tensor("out", [D, S], bf16, kind="ExternalOutput")

    NQ = 4            # s-chunks of 512
    KD = 8            # d contraction tiles
    NS = S // 128     # 16 s-tiles of 128

    with tile.TileContext(nc) as tc:
        with tc.tile_pool(name="const", bufs=1) as const, \
             tc.tile_pool(name="ps", bufs=1, space="PSUM") as ps, \
             tc.tile_pool(name="expp", bufs=4) as expp, \
             tc.tile_pool(name="smalls", bufs=6) as smalls, \
             tc.tile_pool(name="outp", bufs=4) as outp:
          for _rep in range(n_reps):
            xt_sb = const.tile([128, KD, S], bf16)
            wqk_sb = const.tile([128, KD, 2 * EL], bf16)
            wv_sb = const.tile([128, KD, EL], bf16)
            wo_sb = const.tile([128, 2, D], f32r)
            qkT = const.tile([128, 4, S], bf16)
            vhat = const.tile([128, NS, HL * (HD + 1)], bf16)
            attnT = const.tile([128, 2, S], f32r)

            for j in range(KD):
                nc.sync.dma_start(out=xt_sb[:, j, :], in_=xt[j * 128:(j + 1) * 128, :])
                nc.sync.dma_start(out=wqk_sb[:, j, :], in_=wqk[j * 128:(j + 1) * 128, :])
            for j in range(KD):
                nc.sync.dma_start(out=wv_sb[:, j, :], in_=wv[j * 128:(j + 1) * 128, :])
            for j in range(2):
                nc.sync.dma_start(out=wo_sb[:, j, :], in_=wo[j * 128:(j + 1) * 128, :])

            for h in range(HL):
                c = h * (HD + 1) + HD
                nc.vector.memset(vhat[:, :, c:c + 1], 1.0)

            def emit_v(mv):
                p = ps.tile([128, 512], f32, tag="p", bufs=2)
                for k in range(KD):
                    nc.tensor.matmul(
                        p[:, 0:EL],
                        xt_sb[:, k, mv * 128:(mv + 1) * 128],
                        wv_sb[:, k, :],
                        start=(k == 0), stop=(k == KD - 1))
                for h in range(HL):
                    nc.vector.tensor_copy(
                        vhat[:, mv, h * (HD + 1):h * (HD + 1) + HD],
                        p[:, h * HD:(h + 1) * HD])

            _qk_p = {}

            def emit_qk_a(m, n):
                p = ps.tile([128, 512], f32, tag="p", bufs=2)
                _qk_p[(m, n)] = p
                for k in range(0, 4):
                    nc.tensor.matmul(
                        p,
                        wqk_sb[:, k, m * 128:(m + 1) * 128],
                        xt_sb[:, k, n * 512:(n + 1) * 512],
                        start=(k == 0), stop=False)

            def emit_qk_b(m, n):
                p = _qk_p.pop((m, n))
                for k in range(4, KD):
                    nc.tensor.matmul(
                        p,
                        wqk_sb[:, k, m * 128:(m + 1) * 128],
                        xt_sb[:, k, n * 512:(n + 1) * 512],
                        start=False, stop=(k == KD - 1))
                nc.vector.tensor_copy(qkT[:, m, n * 512:(n + 1) * 512], p)

            def emit_qk(m, n):
                emit_qk_a(m, n)
                emit_qk_b(m, n)

            def emit_slab(q, hp, fillers, po=None):
                qs = slice(q * 512, (q + 1) * 512)
                ca = (2 * hp) * (HD + 1)
                cb = (2 * hp + 1) * (HD + 1)
                poA = ps.tile([HD + 1, 512], f32, tag="poA", bufs=1)
                poB = ps.tile([HD + 1, 512], f32, tag="poB", bufs=1)
                for kt in range(NS):
                    ks = slice(kt * 128, (kt + 1) * 128)
                    sp = ps.tile([128, 1024], f32, tag="sp", bufs=2)
                    nc.tensor.matmul(
                        sp[:, 0:512],
                        qkT[0:64, 2 + hp, ks], qkT[0:64, hp, qs],
                        start=True, stop=True, tile_position=(0, 0))
                    nc.tensor.matmul(
                        sp[:, 512:1024],
                        qkT[64:128, 2 + hp, ks], qkT[64:128, hp, qs],
                        start=True, stop=True, tile_position=(64, 0))
                    et = expp.tile([128, 1024], bf16)
                    nc.scalar.activation(et, sp, AF.Exp, scale=SCALE)
                    # filler between exp and PV: PE chews this while ACT
                    # computes the exp, so PV finds its et ready.
                    for f in fillers.get(kt, []):
                        f()
                    nc.tensor.matmul(
                        poA, vhat[:, kt, ca:ca + HD + 1], et[:, 0:512],
                        start=(kt == 0), stop=(kt == NS - 1))
                    nc.tensor.matmul(
                        poB, vhat[:, kt, cb:cb + HD + 1], et[:, 512:1024],
                        start=(kt == 0), stop=(kt == NS - 1))
                # stage po to SBUF with one copy each so the PSUM banks free
                # immediately; normalize off the staged copy.
                stA = smalls.tile([HD + 1, 512], f32)
                stB = smalls.tile([HD + 1, 512], f32)
                nc.vector.tensor_copy(stA, poA)
                nc.vector.tensor_copy(stB, poB)
                recA = smalls.tile([1, 512], f32)
                recB = smalls.tile([1, 512], f32)
                nc.vector.reciprocal(recA, stA[64:65, :])
                nc.vector.reciprocal(recB, stB[64:65, :])
                bcA = smalls.tile([64, 512], f32)
                bcB = smalls.tile([64, 512], f32)
                nc.gpsimd.partition_broadcast(bcA, recA)
                nc.gpsimd.partition_broadcast(bcB, recB)
                nc.vector.tensor_mul(attnT[0:64, hp, qs], stA[0:64, :], bcA)
                nc.vector.tensor_mul(attnT[64:128, hp, qs], stB[0:64, :], bcB)

            def emit_outproj(q, ms=range(8)):
                qs = slice(q * 512, (q + 1) * 512)
                for m in ms:
                    p = ps.tile([128, 512], f32, tag="p", bufs=2)
                    nc.tensor.matmul(
                        p, wo_sb[:, 0, m * 128:(m + 1) * 128],
                        attnT[:, 0, qs], start=True, stop=False)
                    nc.tensor.matmul(
                        p, wo_sb[:, 1, m * 128:(m + 1) * 128],
                        attnT[:, 1, qs], start=False, stop=True)
                    ot = outp.tile([128, 512], bf16)
                    nc.vector.tensor_copy(ot, p)
                    nc.sync.dma_start(
                        out=out[m * 128:(m + 1) * 128, q * 512:(q + 1) * 512],
                        in_=ot)

            # Per-kt interleave [scores | filler | PV]; PE never waits on
            # exp, ACT stays saturated.  Filler deadlines:
            #   qk(2+hp, n): before slab (.,hp) scores kt=4n  (slot <= 4n-1)
            #   qk(hp, q): before slab (q, hp)
            #   v(k): before PV of kt=k in the first slab (slot <= k)
            #   outproj(q): after both (q,.) slabs
            emit_qk(2, 0)
            emit_qk(0, 0)
            for mv in range(4):
                emit_v(mv)

            # slab 1: (0,0)
            emit_slab(0, 0, {
                0: [lambda: emit_qk_a(2, 1)],
                1: [lambda: emit_qk_b(2, 1)],
                2: [lambda: emit_v(4)],
                3: [lambda: emit_v(5)],
                4: [lambda: emit_qk_a(2, 2), lambda: emit_v(6)],
                5: [lambda: emit_qk_b(2, 2)],
                6: [lambda: emit_v(7)],
                7: [lambda: emit_v(8)],
                8: [lambda: emit_qk_a(2, 3), lambda: emit_v(9)],
                9: [lambda: emit_qk_b(2, 3), lambda: emit_v(10)],
                10: [lambda: emit_v(11)],
                11: [lambda: emit_v(12)],
                12: [lambda: emit_v(13)],
                13: [lambda: emit_v(14)],
                14: [lambda: emit_v(15)],
                15: [lambda: emit_qk_a(3, 0), lambda: emit_qk_b(3, 0),
                     lambda: emit_qk_a(1, 0), lambda: emit_qk_b(1, 0)],
            })
            # slab 2: (0,1)
            emit_slab(0, 1, {
                2: [lambda: emit_qk_a(3, 1)],
                3: [lambda: emit_qk_b(3, 1)],
                6: [lambda: emit_qk_a(3, 2)],
                7: [lambda: emit_qk_b(3, 2)],
                10: [lambda: emit_qk_a(3, 3)],
                11: [lambda: emit_qk_b(3, 3)],
                13: [lambda: emit_qk_a(0, 1)],
                14: [lambda: emit_qk_b(0, 1)],
            })
            # slab 3: (1,0)
            emit_slab(1, 0, {
                1: [lambda: emit_qk_a(1, 1)],
                2: [lambda: emit_qk_b(1, 1)],
                4: [lambda: emit_outproj(0, range(0, 2))],
                6: [lambda: emit_outproj(0, range(2, 4))],
                8: [lambda: emit_outproj(0, range(4, 6))],
                10: [lambda: emit_outproj(0, range(6, 8))],
                12: [lambda: emit_qk_a(0, 2)],
                13: [lambda: emit_qk_b(0, 2)],
            })
            # slab 4: (1,1)
            emit_slab(1, 1, {
                5: [lambda: emit_qk_a(1, 2)],
                6: [lambda: emit_qk_b(1, 2)],
                11: [lambda: emit_qk_a(0, 3)],
                12: [lambda: emit_qk_b(0, 3)],
            })
            # slab 5: (2,0)
            emit_slab(2, 0, {
                2: [lambda: emit_outproj(1, range(0, 2))],
                4: [lambda: emit_outproj(1, range(2, 4))],
                7: [lambda: emit_outproj(1, range(4, 6))],
                9: [lambda: emit_outproj(1, range(6, 8))],
                12: [lambda: emit_qk_a(1, 3)],
                13: [lambda: emit_qk_b(1, 3)],
            })
            # slab 6: (2,1)
            emit_slab(2, 1, {})
            # slab 7: (3,0)
            emit_slab(3, 0, {
                2: [lambda: emit_outproj(2, range(0, 2))],
                5: [lambda: emit_outproj(2, range(2, 4))],
                8: [lambda: emit_outproj(2, range(4, 6))],
                11: [lambda: emit_outproj(2, range(6, 8))],
            })
            # slab 8: (3,1)
            emit_slab(3, 1, {})
            emit_outproj(3, range(0, 8))

    nc.finalize()
    return nc


def make_in_maps(x, w_qkv, w_out):
    x = np.asarray(x, dtype=np.float32)
    w_qkv = np.asarray(w_qkv, dtype=np.float32)
    w_out = np.asarray(w_out, dtype=np.float32)
    in_maps = []
    for c in range(N_CORES):
        b, g = divmod(c, 4)
        r0 = g * EL
        wq = w_qkv[r0:r0 + EL]
        wk = w_qkv[D + r0:D + r0 + EL]
        wv_ = w_qkv[2 * D + r0:2 * D + r0 + EL]
        in_maps.append({
            "xt": np.ascontiguousarray(x[b].T).astype(ml_dtypes.bfloat16),
            "wqk": np.ascontiguousarray(np.concatenate([wq, wk], axis=0).T).astype(ml_dtypes.bfloat16),
            "wv": np.ascontiguousarray(wv_.T).astype(ml_dtypes.bfloat16),
            "wo": np.ascontiguousarray(w_out[:, r0:r0 + EL].T),
        })
    return in_maps


def kernel(x, w_qkv, w_out):
    if "nc" not in _CACHE:
        _CACHE["nc"] = build_nc()
    nc = _CACHE["nc"]
    in_maps = make_in_maps(x, w_qkv, w_out)
    res = run_bass_kernel_spmd(nc, in_maps, core_ids=list(range(N_CORES)))
    outs = [res.results[c]["out"] for c in range(N_CORES)]
    final = np.empty((B, S, D), dtype=np.float32)
    for b in range(B):
        acc = outs[4 * b].astype(np.float32)
        for g in range(1, 4):
            acc += outs[4 * b + g].astype(np.float32)
        final[b] = acc.T
    return final

